# revision 1
# baseline (speedup 1.0000x reference)
"""BiologicallyInformedAttention TRN2 kernel (8 NeuronCores, axon/PJRT).

Sharding: B*H = 32 (batch, head) pairs over 8 cores -> core c handles batch
c//2, heads (c%2)*4 .. +4. Projection weights are column-sliced per core; x is
transposed host-side so every matmul contracts over the partition dim.

Per-core dataflow (fp32r matmuls = full PE rate at N>=256, ~1.5e-4 rel err):
  qT/kT = W.T @ xT -> [hd, s] f32r   (bias fused into DVE psum evacuation)
  v     = xT.T @ Wv -> per-head [v|1|1] 66-col groups (bias via K=1 matmul)
  per head-pair, per q-half (1024):
    scoresT[k,q] psum: 2 heads packed via tile_position row groups (K=64 each)
    prior: += 8*pw*eye via PE accumulate matmul (shifted-identity const)
    exp on ACT (scale=1/sqrt(dh) fused) -> f32r ET tiles
    AV: av[66,1024] += [v|1|1].T @ ET   (row 64 = softmax denominators)
    normalize: recip_approx(sums) -> gpsimd partition_broadcast -> DVE mul
  outT[64, s] = sum_h Wo_h.T @ attnT_h  (emitted as soon as a q-half is done)
Host: out[b] = (outT[2b] + outT[2b+1]).T + bo.
"""
import os
import numpy as np
from contextlib import ExitStack

import concourse.bacc as bacc
import concourse.tile as tile
from concourse import mybir
from concourse.bass_utils import run_bass_kernel_spmd

B, S, D, H, DH = 4, 2048, 512, 8, 64
HPC = H // 2          # heads per core = 4
W_COLS = HPC * DH     # 256 per-core projection columns
N_CORES = 8

f32 = mybir.dt.float32
f32r = mybir.dt.float32r
AF = mybir.ActivationFunctionType

_BUILT = {}


def _build(repeat=1):
    nc = bacc.Bacc("TRN2", target_bir_lowering=False)

    xT_d = nc.declare_dram_parameter("xT", [D, S], f32r, isOutput=False)
    wq_d = nc.declare_dram_parameter("wq", [D, W_COLS], f32r, isOutput=False)
    wk_d = nc.declare_dram_parameter("wk", [D, W_COLS], f32r, isOutput=False)
    wv_d = nc.declare_dram_parameter("wv", [D, W_COLS], f32r, isOutput=False)
    wo_d = nc.declare_dram_parameter("wo", [W_COLS, DH], f32r, isOutput=False)
    bq_d = nc.declare_dram_parameter("bq", [W_COLS, 1], f32, isOutput=False)
    bk_d = nc.declare_dram_parameter("bk", [W_COLS, 1], f32, isOutput=False)
    bv_d = nc.declare_dram_parameter("bv", [1, W_COLS], f32r, isOutput=False)
    pw8i_d = nc.declare_dram_parameter("pw8i", [128, 128], f32, isOutput=False)
    ones_row_d = nc.declare_dram_parameter("ones_row", [1, 128], f32r, isOutput=False)
    ones_blk_d = nc.declare_dram_parameter("ones_blk", [128, 8], f32r, isOutput=False)
    outT_d = nc.declare_dram_parameter("outT", [DH, S], f32, isOutput=True)

    with tile.TileContext(nc) as tc, ExitStack() as ctx:
        cp = ctx.enter_context(tc.tile_pool(name="cp", bufs=1))

        # ---------- persistent tiles ----------
        xr = [cp.tile([128, S], f32r, tag=f"xr{i}", name=f"xr{i}") for i in range(4)]
        wqr = [cp.tile([128, W_COLS], f32r, tag=f"wqr{i}", name=f"wqr{i}") for i in range(4)]
        wkr = [cp.tile([128, W_COLS], f32r, tag=f"wkr{i}", name=f"wkr{i}") for i in range(4)]
        wvr = [cp.tile([128, W_COLS], f32r, tag=f"wvr{i}", name=f"wvr{i}") for i in range(4)]
        wor = cp.tile([DH, W_COLS], f32r, tag="wor", name="wor")
        bq_t = cp.tile([128, 2], f32, tag="bq", name="bq")
        bk_t = cp.tile([128, 2], f32, tag="bk", name="bk")
        bvr = cp.tile([1, W_COLS], f32r, tag="bvr", name="bvr")
        ones_col = cp.tile([1, 128], f32r, tag="ones_col", name="ones_col")
        ones_blk = cp.tile([128, 8], f32r, tag="ones_blk", name="ones_blk")
        pw8i = cp.tile([128, 128], f32, tag="pw8i", name="pw8i")
        qTr = [cp.tile([128, S], f32r, tag=f"qTr{p}", name=f"qTr{p}") for p in range(2)]
        kTr = [cp.tile([128, S], f32r, tag=f"kTr{p}", name=f"kTr{p}") for p in range(2)]
        v_aug = [cp.tile([128, HPC * 66], f32r, tag=f"va{st}", name=f"va{st}") for st in range(16)]
        attnT = [cp.tile([DH, S], f32r, tag=f"at{h}", name=f"at{h}") for h in range(HPC)]
        outT_s = cp.tile([DH, S], f32, tag="outT", name="outT")

        # ---------- loads (all f32r DMA-direct) ----------
        for di in range(4):
            nc.sync.dma_start(wqr[di][:], wq_d[di * 128:(di + 1) * 128, :])
            nc.sync.dma_start(wkr[di][:], wk_d[di * 128:(di + 1) * 128, :])
        for sc4 in range(4):
            s0 = sc4 * 512
            for di in range(4):
                nc.sync.dma_start(xr[di][:, s0:s0 + 512],
                                  xT_d[di * 128:(di + 1) * 128, s0:s0 + 512])
            if sc4 == 1:
                for di in range(4):
                    nc.sync.dma_start(wvr[di][:], wv_d[di * 128:(di + 1) * 128, :])
        for h in range(HPC):
            nc.sync.dma_start(wor[:, h * DH:(h + 1) * DH],
                              wo_d[h * DH:(h + 1) * DH, :])
        for ht in range(2):
            nc.sync.dma_start(bq_t[:, ht:ht + 1], bq_d[ht * 128:(ht + 1) * 128, :])
            nc.sync.dma_start(bk_t[:, ht:ht + 1], bk_d[ht * 128:(ht + 1) * 128, :])
        nc.sync.dma_start(bvr[:], bv_d[:])
        nc.sync.dma_start(ones_col[:], ones_row_d[:])
        nc.sync.dma_start(ones_blk[:], ones_blk_d[:])
        nc.sync.dma_start(pw8i[:], pw8i_d[:])

        def proj_qk(ht, sc4s):
            for sc4 in sc4s:
                s0 = sc4 * 512
                for wr, bias_t, dst in ((wqr, bq_t, qTr), (wkr, bk_t, kTr)):
                    pt = scp.tile([128, 512], f32, tag="sc", name="pj")
                    for di in range(4):
                        nc.tensor.matmul(
                            pt[:],
                            wr[di][:, ht * 128:(ht + 1) * 128],
                            xr[di][:, s0:s0 + 512],
                            start=(di == 0), stop=(di == 3))
                    nc.vector.tensor_scalar_add(
                        dst[ht][:, s0:s0 + 512], pt[:], bias_t[:, ht:ht + 1])

        def proj_v(sts):
            for st in sts:
                pv = scp.tile([128, W_COLS], f32, tag="sc", name="pv")
                for di in range(4):
                    nc.tensor.matmul(pv[:],
                                     xr[di][:, st * 128:(st + 1) * 128],
                                     wvr[di][:],
                                     start=(di == 0), stop=False)
                nc.tensor.matmul(pv[:], ones_col[:], bvr[:],
                                 start=False, stop=True)
                va = v_aug[st][:].rearrange("p (h c) -> p h c", c=66)
                nc.vector.tensor_copy(
                    va[:, :, 0:DH],
                    pv[:].rearrange("p (h c) -> p h c", c=DH))
                nc.vector.tensor_copy(
                    va[:, :, DH:66],
                    ones_blk[:].rearrange("p (h c) -> p h c", c=2))

        def attention_pair(p, extras=()):
            extras = list(extras)
            h0, h1 = 2 * p, 2 * p + 1
            for qh in range(2):
                q0 = qh * 1024
                av0 = avp.tile([66, 1024], f32, tag="av0", name="av0")
                av1 = avp.tile([66, 1024], f32, tag="av1", name="av1")
                for kt in range(16):
                    if extras:
                        extras.pop(0)()
                    k0 = kt * 128
                    sc0 = scp.tile([128, 1024], f32, tag="sc", name="sc")
                    sc1 = scp.tile([128, 1024], f32, tag="sc", name="sc")
                    off = k0 - q0
                    for qc in range(2):
                        qq = qc * 512
                        qg = q0 + qq
                        for sc_t, base in ((sc0, 0), (sc1, 64)):
                            nc.tensor.matmul(
                                sc_t[:, qq:qq + 512],
                                kTr[p][base:base + 64, k0:k0 + 128],
                                qTr[p][base:base + 64, qg:qg + 512],
                                start=True, stop=True,
                                tile_position=(base, 0))
                    if 0 <= off < 1024:
                        nc.vector.tensor_add(sc0[:, off:off + 128],
                                             sc0[:, off:off + 128], pw8i[:])
                        nc.vector.tensor_add(sc1[:, off:off + 128],
                                             sc1[:, off:off + 128], pw8i[:])
                    et0 = etp.tile([128, 1024], f32r, tag="et", name="et")
                    et1 = etp.tile([128, 1024], f32r, tag="et", name="et")
                    nc.scalar.activation(et0[:], sc0[:], AF.Exp, scale=0.125)
                    nc.scalar.activation(et1[:], sc1[:], AF.Exp, scale=0.125)
                    for qc in range(2):
                        qq = qc * 512
                        nc.tensor.matmul(
                            av0[:, qq:qq + 512],
                            v_aug[kt][:, h0 * 66:h0 * 66 + 66],
                            et0[:, qq:qq + 512],
                            start=(kt == 0), stop=(kt == 15))
                        nc.tensor.matmul(
                            av1[:, qq:qq + 512],
                            v_aug[kt][:, h1 * 66:h1 * 66 + 66],
                            et1[:, qq:qq + 512],
                            start=(kt == 0), stop=(kt == 15))
                for hh, av in ((h0, av0), (h1, av1)):
                    avs = nrm.tile([66, 1024], f32, tag="avs", name="avs", bufs=2)
                    nc.vector.tensor_copy(avs[:], av[:])
                    sums = nrm.tile([1, 1024], f32, tag="sums", name="sums", bufs=2)
                    nc.vector.tensor_copy(sums[:], avs[DH:DH + 1, :])
                    recip = nrm.tile([1, 1024], f32, tag="recip", name="recip")
                    nc.vector.reciprocal_approx_fast(recip[:], sums[:])
                    rB = nrm.tile([DH, 1024], f32, tag="rB", name="rB", bufs=2)
                    nc.gpsimd.partition_broadcast(rB[:], recip[:])
                    nc.vector.tensor_mul(attnT[hh][:, q0:q0 + 1024],
                                         avs[0:DH, :], rB[:])
                if p == 1:
                    out_proj(qh)

        def out_proj(qh):
            for sc4 in (2 * qh, 2 * qh + 1):
                s0 = sc4 * 512
                po = (avp.tile([DH, 512], f32, tag="av0", name="po")
                      if os.environ.get("KPOAV")
                      else scp.tile([DH, 512], f32, tag="sc", name="po"))
                for h in range(HPC):
                    nc.tensor.matmul(po[:],
                                     wor[:, h * DH:(h + 1) * DH],
                                     attnT[h][:, s0:s0 + 512],
                                     start=(h == 0), stop=(h == HPC - 1))
                nc.vector.tensor_copy(outT_s[:, s0:s0 + 512], po[:])
                nc.sync.dma_start(outT_d[:, s0:s0 + 512], outT_s[:, s0:s0 + 512])

        with tc.tile_pool(name="scp", bufs=2, space="PSUM") as scp, \
             tc.tile_pool(name="avp", bufs=1, space="PSUM") as avp, \
             tc.tile_pool(name="etp", bufs=int(os.environ.get("KETBUFS", "6"))) as etp, \
             tc.tile_pool(name="nrm", bufs=1) as nrm:
            for _rep in range(repeat):
                proj_qk(0, range(2))
                proj_v(range(4))
                proj_qk(0, range(2, 4))
                if os.environ.get("KEXTRAS"):
                    extras = [(lambda st=st: proj_v([st])) for st in range(4, 16)]
                    extras += [(lambda s=s: proj_qk(1, [s])) for s in range(4)]
                    attention_pair(0, extras)
                else:
                    proj_v(range(4, 16))
                    proj_qk(1, range(4))
                    attention_pair(0)
                attention_pair(1)

    nc.finalize()
    return nc


def _get_nc(repeat=1):
    if repeat not in _BUILT:
        _BUILT[repeat] = _build(repeat)
    return _BUILT[repeat]


def _make_in_maps(x, Wq, bq, Wk, bk, Wv, bv, Wo, bo, prior_weight):
    pw8i = (8.0 * float(prior_weight[0])) * np.eye(128, dtype=np.float32)
    ones_row = np.ones((1, 128), np.float32)
    ones_blk = np.ones((128, 8), np.float32)
    xT = [np.ascontiguousarray(x[b].T) for b in range(B)]
    in_maps = []
    for c in range(N_CORES):
        b, half = c // 2, c % 2
        cs = slice(half * W_COLS, (half + 1) * W_COLS)
        in_maps.append({
            "xT": xT[b],
            "wq": np.ascontiguousarray(Wq[:, cs]),
            "wk": np.ascontiguousarray(Wk[:, cs]),
            "wv": np.ascontiguousarray(Wv[:, cs]),
            "wo": np.ascontiguousarray(Wo[cs, :]),
            "bq": np.ascontiguousarray(bq[cs].reshape(W_COLS, 1)),
            "bk": np.ascontiguousarray(bk[cs].reshape(W_COLS, 1)),
            "bv": np.ascontiguousarray(bv[cs].reshape(1, W_COLS)),
            "pw8i": pw8i,
            "ones_row": ones_row,
            "ones_blk": ones_blk,
        })
    return in_maps


def run(inputs, trace=False, trace_cores=None):
    """Execute on 8 cores; returns (output [B,S,DH] f32, BassKernelResults)."""
    args = {k: np.asarray(v) for k, v in inputs.items()}
    nc = _get_nc()
    in_maps = _make_in_maps(
        args["x"], args["Wq"], args["bq"], args["Wk"], args["bk"],
        args["Wv"], args["bv"], args["Wo"], args["bo"], args["prior_weight"])
    res = run_bass_kernel_spmd(
        nc, in_maps, list(range(N_CORES)), trace=trace,
        **({"trace_cores": trace_cores} if trace_cores else {}))
    bo = args["bo"].astype(np.float32)
    out = np.empty((B, S, DH), np.float32)
    for b in range(B):
        acc = res.results[2 * b]["outT"] + res.results[2 * b + 1]["outT"]
        out[b] = acc.T + bo
    return out, res


def kernel(**inputs) -> np.ndarray:
    out, _ = run(inputs, trace=False)
    return out

